# revision 8
# baseline (speedup 1.0000x reference)
"""Grouped-Query Attention (B=2,S=2048,DIN=2048,H=32,G=8,D=64) on 8 trn2 cores.

Sharding: tensor-parallel over KV groups — core g owns KV group g (4 query
heads, both batches). Wq/Wk/Wv column-sharded, Wo row-sharded; per-core
partial outputs summed on host. k/v returned per-group and stacked on host.

Device algorithm per core:
  A) per (batch, 128-row s-tile): transpose x slab on PE, project q/k/v,
     qk-RMSNorm + RoPE (scales folded into host-precomputed cos/sin tables),
     transpose q/k to [d, s] layout for attention.
  B) flash-style attention with TRANSPOSED scores sT[k, q] = kT.T @ qT so the
     softmax denominator comes free as an extra ones-column in the ctx matmul
     (out rows = [v | 1]). No max-subtraction needed: |scores| <= 8 post-RMSNorm.
     Causal masking by skipping fully-masked k-chunks and multiplying the 4
     boundary chunks by a precomputed 0/1 ramp.
  C) out_partial = ctx^T pairs @ Wo rows, streamed to DRAM.
"""

import os
import sys
import types

import numpy as np

B, S, DIN = 2, 2048, 2048
H, G, D = 32, 8, 64
GS = H // G          # 4 query heads per core
P = 128
EPS = 1e-6
NT = S // P          # 16 s-tiles per batch
NCH = DIN // P       # 16 contraction chunks
QF = 512             # q-column chunk in attention
NQC = S // QF        # 4
NC_ = 8

CDT_NAME = os.environ.get("BASS_CDT", "float32r")
LAST_EXEC_NS = [None]

_cache = {}


def _install_ntff_hook():
    if "antenv.axon_hooks" in sys.modules:
        return
    try:
        import antenv  # noqa: F401
    except ImportError:
        return
    mod = types.ModuleType("antenv.axon_hooks")
    holder = [None]
    mod.set_axon_ntff_profile_hook = lambda h: holder.__setitem__(0, h)
    mod.get_axon_ntff_profile_hook = lambda: holder[0]
    sys.modules["antenv.axon_hooks"] = mod
    try:
        boot_dir = "/root/.axon_site/trn_agent_boot"
        if boot_dir not in sys.path and os.path.isdir(boot_dir):
            sys.path.insert(0, boot_dir)
        import trn_boot
        so = "/opt/axon/libaxon_pjrt.so"
        if os.path.exists(so):
            mod.set_axon_ntff_profile_hook(trn_boot._ntff_profile_via_ctypes(so))
    except Exception:
        pass


def _build(cdt_name):
    import concourse.bass as bass  # noqa: F401
    import concourse.mybir as mybir
    from concourse import bacc
    from concourse.tile import TileContext
    from concourse.masks import make_identity

    f32 = mybir.dt.float32
    CDT = getattr(mybir.dt, cdt_name)
    AF = mybir.ActivationFunctionType
    cast_needed = cdt_name != "float32"

    nc = bacc.Bacc("TRN2", target_bir_lowering=False, debug=False, num_devices=NC_)

    x_d = nc.dram_tensor("x", [B, S, DIN], f32, kind="ExternalInput")
    wq_d = nc.dram_tensor("wq", [DIN, GS * D], f32, kind="ExternalInput")
    wkv_d = nc.dram_tensor("wkv", [DIN, 2 * D], f32, kind="ExternalInput")
    wo_d = nc.dram_tensor("wo", [GS * D, DIN], f32, kind="ExternalInput")
    cosq_d = nc.dram_tensor("cosq", [S, D], f32, kind="ExternalInput")
    cosk_d = nc.dram_tensor("cosk", [S, D], f32, kind="ExternalInput")
    nsinq_d = nc.dram_tensor("nsinq", [S, D // 2], f32, kind="ExternalInput")
    sinq2_d = nc.dram_tensor("sinq2", [S, D // 2], f32, kind="ExternalInput")
    nsink_d = nc.dram_tensor("nsink", [S, D // 2], f32, kind="ExternalInput")
    sink2_d = nc.dram_tensor("sink2", [S, D // 2], f32, kind="ExternalInput")
    tri_d = nc.dram_tensor("tri", [P, 896], f32, kind="ExternalInput")
    out_d = nc.dram_tensor("out", [B, S, DIN], f32, kind="ExternalOutput")
    kout_d = nc.dram_tensor("kout", [B, S, D], f32, kind="ExternalOutput")
    vout_d = nc.dram_tensor("vout", [B, S, D], f32, kind="ExternalOutput")

    from contextlib import ExitStack
    with TileContext(nc) as tc, ExitStack() as es:
        consts = es.enter_context(tc.tile_pool(name="consts", bufs=1))
        qt_pool = es.enter_context(tc.tile_pool(name="qt", bufs=4))
        kt_pool = es.enter_context(tc.tile_pool(name="kt", bufs=1))
        v_pool = es.enter_context(tc.tile_pool(name="v", bufs=2))
        ct_pool = es.enter_context(tc.tile_pool(name="ct", bufs=4))
        xs_pool = es.enter_context(tc.tile_pool(name="xs", bufs=2))
        xt_pool = es.enter_context(tc.tile_pool(name="xt", bufs=2))
        st_pool = es.enter_context(tc.tile_pool(name="stg", bufs=2))
        e_pool = es.enter_context(tc.tile_pool(name="e", bufs=3))
        o_pool = es.enter_context(tc.tile_pool(name="o", bufs=3))
        mm_ps = es.enter_context(tc.tile_pool(name="mmps", bufs=2, space="PSUM"))
        qp_ps = es.enter_context(tc.tile_pool(name="qpps", bufs=1, space="PSUM"))
        kv_ps = es.enter_context(tc.tile_pool(name="kvps", bufs=1, space="PSUM"))
        sT_ps = es.enter_context(tc.tile_pool(name="stps", bufs=2, space="PSUM"))
        cx_ps = es.enter_context(tc.tile_pool(name="cxps", bufs=2, space="PSUM"))

        # ---- constants / weights ----
        def load_w(dram_ap, shape, nm):
            t = consts.tile(shape, CDT if cast_needed else f32, tag=nm, name=nm)
            if not cast_needed:
                nc.sync.dma_start(out=t, in_=dram_ap)
                return t
            # chunked stage+cast to keep the staging footprint at 2x512 f32
            step = 512
            if len(shape) == 2:
                pieces = [(t[:, c0:min(c0 + step, shape[1])],
                           dram_ap[:, c0:min(c0 + step, shape[1])])
                          for c0 in range(0, shape[1], step)]
            else:
                pieces = []
                for a in range(shape[1]):
                    for c0 in range(0, shape[2], step):
                        c1 = min(c0 + step, shape[2])
                        pieces.append((t[:, a, c0:c1], dram_ap[:, a, c0:c1]))
            for i, (dst, srcap) in enumerate(pieces):
                w = dst.shape[-1]
                stage = consts.tile([P, step], f32, tag="wstage", bufs=2,
                                    name=nm + f"_st{i}")
                nc.sync.dma_start(out=stage[:, 0:w], in_=srcap)
                nc.vector.tensor_copy(dst, stage[:, 0:w])
            return t

        wq_sb = load_w(wq_d.ap().rearrange("(c p) m -> p c m", p=P), [P, NCH, GS * D], "wqsb")
        wkv_sb = load_w(wkv_d.ap().rearrange("(c p) m -> p c m", p=P), [P, NCH, 2 * D], "wkvsb")
        wo_sb = load_w(wo_d.ap().rearrange("(r p) n -> p r n", p=P), [P, 2, DIN], "wosb")
        tri_sb = load_w(tri_d.ap(), [P, 896], "trisb")

        def load_c(dram_ap, cols, nm):
            t = consts.tile([P, NT, cols], f32, tag=nm, name=nm)
            nc.sync.dma_start(out=t, in_=dram_ap.rearrange("(t p) d -> p t d", p=P))
            return t

        cosq_sb = load_c(cosq_d.ap(), D, "cosqsb")
        cosk_sb = load_c(cosk_d.ap(), D, "cosksb")
        nsinq_sb = load_c(nsinq_d.ap(), D // 2, "nsinqsb")
        sinq2_sb = load_c(sinq2_d.ap(), D // 2, "sinq2sb")
        nsink_sb = load_c(nsink_d.ap(), D // 2, "nsinksb")
        sink2_sb = load_c(sink2_d.ap(), D // 2, "sink2sb")

        ident = consts.tile([P, P], f32)
        make_identity(nc, ident)
        ones_f = consts.tile([P, 1], f32)
        nc.vector.memset(ones_f, 1.0)
        eps_sb = consts.tile([P, 1], f32)
        nc.vector.memset(eps_sb, EPS)

        # persistent per-batch tensors
        qT = [qt_pool.tile([P, S], CDT, tag="qt", name=f"qTh{h2}") for h2 in range(GS)]
        kT = kt_pool.tile([P, S], CDT)
        v_sb = [v_pool.tile([P, NT, D + 1], CDT, tag="v", name=f"vsb{bb}") for bb in range(B)]
        ctxT = [[ct_pool.tile([P, S], CDT, tag="ct", name=f"ctxT{bb}{pp}") for pp in range(2)] for bb in range(B)]

        def phase_a(b):
            for t in range(NT):
                xs = xs_pool.tile([P, DIN], f32, tag="xs")
                nc.sync.dma_start(out=xs, in_=x_d[b, t * P:(t + 1) * P, :])
                # transpose slab: xT[:, c, :] = xs[:, c*128:(c+1)*128].T
                xT = xt_pool.tile([P, NCH, P], CDT, tag="xt")
                for c in range(NCH):
                    tp = mm_ps.tile([P, 512], f32, tag="mm")
                    nc.tensor.transpose(tp[:, 0:P], xs[:, c * P:(c + 1) * P], ident)
                    nc.vector.tensor_copy(xT[:, c, :], tp[:, 0:P])
                # projections
                qp = qp_ps.tile([P, GS * D], f32, tag="qp")
                for c in range(NCH):
                    nc.tensor.matmul(qp, lhsT=xT[:, c, :], rhs=wq_sb[:, c, :],
                                     start=(c == 0), stop=(c == NCH - 1))
                qn = st_pool.tile([P, GS * D], f32, tag="qn")
                nc.vector.tensor_copy(qn, qp)
                kvp = kv_ps.tile([P, 2 * D], f32, tag="kvp")
                for c in range(NCH):
                    nc.tensor.matmul(kvp, lhsT=xT[:, c, :], rhs=wkv_sb[:, c, :],
                                     start=(c == 0), stop=(c == NCH - 1))
                kvn = st_pool.tile([P, 2 * D], f32, tag="kvn")
                nc.vector.tensor_copy(kvn, kvp)
                nc.sync.dma_start(out=vout_d[b, t * P:(t + 1) * P, :], in_=kvn[:, D:2 * D])
                nc.vector.tensor_copy(v_sb[b][:, t, 0:D], kvn[:, D:2 * D])
                nc.vector.tensor_copy(v_sb[b][:, t, D:D + 1], ones_f)

                # rmsnorm stats: ms[:, 0:4] per q head, ms[:, 4] for k
                sq = st_pool.tile([P, GS * D], f32, tag="sq")
                nc.vector.tensor_mul(sq, qn, qn)
                ms = st_pool.tile([P, 8], f32, tag="ms")
                nc.vector.tensor_reduce(
                    ms[:, 0:GS], sq.rearrange("p (h d) -> p h d", d=D),
                    axis=mybir.AxisListType.X, op=mybir.AluOpType.add)
                sqk = st_pool.tile([P, D], f32, tag="sqk")
                nc.vector.tensor_mul(sqk, kvn[:, 0:D], kvn[:, 0:D])
                nc.vector.tensor_reduce(ms[:, GS:GS + 1], sqk,
                                        axis=mybir.AxisListType.X, op=mybir.AluOpType.add)
                srt = st_pool.tile([P, 8], f32, tag="srt")
                nc.scalar.activation(srt[:, 0:GS + 1], ms[:, 0:GS + 1], AF.Sqrt,
                                     scale=1.0 / D, bias=eps_sb[:, 0:1])
                rinv = st_pool.tile([P, 8], f32, tag="rinv")
                nc.vector.reciprocal(rinv[:, 0:GS + 1], srt[:, 0:GS + 1])

                # rope q (tables already carry per-d scale): out = xs*cos + rot(xs)*sin
                qr = st_pool.tile([P, GS * D], f32, tag="qr")
                for h in range(GS):
                    qs = st_pool.tile([P, D], f32, tag="qs")
                    nc.vector.tensor_scalar_mul(qs, qn[:, h * D:(h + 1) * D], rinv[:, h:h + 1])
                    m = st_pool.tile([P, D], f32, tag="ropem")
                    nc.vector.tensor_mul(m, qs, cosq_sb[:, t, :])
                    r = st_pool.tile([P, D], f32, tag="roper")
                    nc.vector.tensor_mul(r[:, 0:D // 2], qs[:, D // 2:D], nsinq_sb[:, t, :])
                    nc.vector.tensor_mul(r[:, D // 2:D], qs[:, 0:D // 2], sinq2_sb[:, t, :])
                    nc.vector.tensor_add(qr[:, h * D:(h + 1) * D], m, r)
                # rope k -> f32 (for kout DMA)
                ks = st_pool.tile([P, D], f32, tag="ks")
                nc.vector.tensor_scalar_mul(ks, kvn[:, 0:D], rinv[:, GS:GS + 1])
                km = st_pool.tile([P, D], f32, tag="km")
                nc.vector.tensor_mul(km, ks, cosk_sb[:, t, :])
                kr = st_pool.tile([P, D], f32, tag="kr")
                nc.vector.tensor_mul(kr[:, 0:D // 2], ks[:, D // 2:D], nsink_sb[:, t, :])
                nc.vector.tensor_mul(kr[:, D // 2:D], ks[:, 0:D // 2], sink2_sb[:, t, :])
                kn = st_pool.tile([P, D], f32, tag="kn")
                nc.vector.tensor_add(kn, km, kr)
                nc.sync.dma_start(out=kout_d[b, t * P:(t + 1) * P, :], in_=kn)

                # transposes to [d, s] layout
                for pr in range(2):
                    tq = mm_ps.tile([P, 512], f32, tag="mm")
                    nc.tensor.transpose(tq[:, 0:P], qr[:, pr * P:(pr + 1) * P], ident)
                    nc.vector.tensor_copy(
                        qT[2 * pr][b * D:(b + 1) * D, t * P:(t + 1) * P], tq[0:D, 0:P])
                    nc.vector.tensor_copy(
                        qT[2 * pr + 1][b * D:(b + 1) * D, t * P:(t + 1) * P], tq[D:P, 0:P])
                tk = mm_ps.tile([P, 512], f32, tag="mm")
                nc.tensor.transpose(tk[0:D, 0:P], kn, ident)
                nc.vector.tensor_copy(kT[b * D:(b + 1) * D, t * P:(t + 1) * P], tk[0:D, 0:P])

        def phase_b(b):
            for qc in range(NQC):
                nkc = 4 * qc + 4
                for pr in range(2):
                    cxs = [cx_ps.tile([P, QF], f32, tag="cx", name=f"cx{hh2}") for hh2 in range(2)]
                    for kc in range(nkc):
                        for hh in range(2):
                            sT = sT_ps.tile([P, QF], f32, tag="st")
                            nc.tensor.matmul(
                                sT,
                                lhsT=kT[b * D:(b + 1) * D, kc * P:(kc + 1) * P],
                                rhs=qT[2 * pr + hh][b * D:(b + 1) * D, qc * QF:(qc + 1) * QF],
                                start=True, stop=True)
                            e = e_pool.tile([P, QF], CDT, tag="e")
                            nc.scalar.activation(e, sT, AF.Exp, scale=0.125)
                            if kc >= 4 * qc:
                                off = 384 - (kc - 4 * qc) * P
                                nc.vector.tensor_mul(e, e, tri_sb[:, off:off + QF])
                            nc.tensor.matmul(
                                cxs[hh][0:D + 1, :], lhsT=v_sb[b][:, kc, :], rhs=e,
                                start=(kc == 0), stop=(kc == nkc - 1),
                                skip_group_check=True)
                    for hh in range(2):
                        rec = st_pool.tile([1, QF], f32, tag="rec")
                        nc.vector.reciprocal(rec, cxs[hh][D:D + 1, :])
                        bc = st_pool.tile([D, QF], f32, tag="bc")
                        nc.gpsimd.partition_broadcast(bc, rec)
                        nc.vector.tensor_mul(
                            ctxT[b][pr][hh * D:(hh + 1) * D, qc * QF:(qc + 1) * QF],
                            cxs[hh][0:D, :], bc)

        def phase_c(b):
            for t in range(NT):
                for j in range(DIN // 512):
                    op = mm_ps.tile([P, 512], f32, tag="mm")
                    for pr in range(2):
                        nc.tensor.matmul(op, lhsT=ctxT[b][pr][:, t * P:(t + 1) * P],
                                         rhs=wo_sb[:, pr, j * 512:(j + 1) * 512],
                                         start=(pr == 0), stop=(pr == 1))
                    ob = o_pool.tile([P, 512], f32, tag="ob")
                    nc.vector.tensor_copy(ob, op)
                    nc.sync.dma_start(out=out_d[b, t * P:(t + 1) * P, j * 512:(j + 1) * 512],
                                      in_=ob)

        phase_a(0)
        phase_a(1)
        phase_b(0)
        phase_c(0)
        phase_b(1)
        phase_c(1)

    nc.compile()
    return nc


def _prep_inputs(x, cos, sin, Wq, Wk, Wv, Wo, q_scale, k_scale):
    cos = np.asarray(cos, np.float32)
    sin = np.asarray(sin, np.float32)
    qs = np.asarray(q_scale, np.float32)
    ks = np.asarray(k_scale, np.float32)
    sh = sin[:, :D // 2]
    tri = (np.arange(896, dtype=np.int64)[None, :] - 384 >=
           np.arange(P, dtype=np.int64)[:, None]).astype(np.float32)
    common = dict(
        x=np.ascontiguousarray(np.asarray(x, np.float32)),
        cosq=np.ascontiguousarray(cos * qs[None, :]),
        cosk=np.ascontiguousarray(cos * ks[None, :]),
        nsinq=np.ascontiguousarray(-sh * qs[None, D // 2:]),
        sinq2=np.ascontiguousarray(sh * qs[None, :D // 2]),
        nsink=np.ascontiguousarray(-sh * ks[None, D // 2:]),
        sink2=np.ascontiguousarray(sh * ks[None, :D // 2]),
        tri=tri,
    )
    Wq = np.asarray(Wq, np.float32)
    Wk = np.asarray(Wk, np.float32)
    Wv = np.asarray(Wv, np.float32)
    Wo = np.asarray(Wo, np.float32)
    maps = []
    for g in range(NC_):
        m = dict(common)
        m["wq"] = np.ascontiguousarray(Wq[:, g * GS * D:(g + 1) * GS * D])
        m["wkv"] = np.ascontiguousarray(
            np.concatenate([Wk[:, g * D:(g + 1) * D], Wv[:, g * D:(g + 1) * D]], axis=1))
        m["wo"] = np.ascontiguousarray(Wo[g * GS * D:(g + 1) * GS * D, :])
        maps.append(m)
    return maps


def kernel(x, mask, cos, sin, Wq, Wk, Wv, Wo, q_scale, k_scale):
    _install_ntff_hook()
    from concourse.bass_utils import run_bass_kernel_spmd

    if CDT_NAME not in _cache:
        _cache[CDT_NAME] = _build(CDT_NAME)
    nc = _cache[CDT_NAME]

    in_maps = _prep_inputs(x, cos, sin, Wq, Wk, Wv, Wo, q_scale, k_scale)
    trace = bool(int(os.environ.get("BASS_KERNEL_TRACE", "0")))
    res = run_bass_kernel_spmd(nc, in_maps, core_ids=list(range(NC_)), trace=trace)
    LAST_EXEC_NS[0] = res.exec_time_ns

    outs = res.results
    out = outs[0]["out"].astype(np.float64)
    for g in range(1, NC_):
        out += outs[g]["out"]
    out = out.astype(np.float32)
    k = np.stack([outs[g]["kout"] for g in range(NC_)], axis=1)
    v = np.stack([outs[g]["vout"] for g in range(NC_)], axis=1)
    return out, (k, v)
